# revision 22
# baseline (speedup 1.0000x reference)
"""CommNet critic forward kernel for 8 trn2 NeuronCores.

Sharding: pure data parallel over the batch dim (B=2048 -> 256 per core).
Weights (<1MB) replicated. The agent-mean communication is within each
sample's 32-agent group, which never crosses a core boundary, so there are
no collectives.

v2 vs v1: the engine loads are rebalanced around the measured busy times
(PE ~150us, Scalar ACT 194us, Vector 168us, GpSimd 114us on a 240us span):
  * fobs is folded into GRU1 host-side (Wfold1 = W_hh @ fobs_W), so the
    GRU1 gate matmuls read the encoder output e directly and h0 never
    needs a dedicated PSUM drain: its bias-add merges into the d1 STT.
  * Scalar keeps only the transcendentals (sigmoid/tanh) plus the tiny
    dec drain; relu moves to Vector (tensor_scalar add+max), and the
    GRU h-updates are split Vector/GpSimd so no engine exceeds ~130us.
  * PE stream (44 matmuls/tile, all N=512 bf16) is the binding floor;
    the emit order interleaves A(t+2) work into B(t) so it never stalls.
"""

import sys

sys.path.insert(0, "/opt/trn_rl_repo")

import ml_dtypes
import numpy as np

import concourse.bacc as bacc
import concourse.mybir as mybir
import concourse.tile as tile
from concourse.bass_utils import run_bass_kernel_spmd

B, A, D, H = 2048, 32, 128, 256
NCORES = 8
B_LOC = B // NCORES          # 256 samples per core
N_LOC = B_LOC * A            # 8192 rows per core
R = 512                      # rows per tile (one PSUM bank of fp32)

F32 = mybir.dt.float32
BF16 = mybir.dt.bfloat16
NP_BF16 = ml_dtypes.bfloat16

AF = mybir.ActivationFunctionType
OP = mybir.AluOpType

S0 = slice(0, 512)
S1 = slice(512, 1024)


def build_nc(n_rows=N_LOC):
    assert n_rows % R == 0
    nt = n_rows // R
    nc = bacc.Bacc("TRN2", target_bir_lowering=False, debug=False)

    xT = nc.declare_dram_parameter("xT", [D, n_rows], BF16, isOutput=False)
    encT = nc.declare_dram_parameter("encT", [128, 256], BF16, isOutput=False)
    fobsT = nc.declare_dram_parameter("fobsT", [128, 512], BF16, isOutput=False)
    wf1T = nc.declare_dram_parameter("wf1T", [128, 1536], BF16, isOutput=False)
    whhT = nc.declare_dram_parameter("whhT", [128, 1536], BF16, isOutput=False)
    wihT = nc.declare_dram_parameter("wihT", [128, 1536], BF16, isOutput=False)
    decT = nc.declare_dram_parameter("decT", [128, 2], BF16, isOutput=False)
    encb = nc.declare_dram_parameter("encb", [128, 2], F32, isOutput=False)
    fobsb = nc.declare_dram_parameter("fobsb", [128, 2], F32, isOutput=False)
    brz1 = nc.declare_dram_parameter("brz1", [128, 4], F32, isOutput=False)
    bhn1 = nc.declare_dram_parameter("bhn1", [128, 2], F32, isOutput=False)
    brz2 = nc.declare_dram_parameter("brz2", [128, 4], F32, isOutput=False)
    bhn2 = nc.declare_dram_parameter("bhn2", [128, 2], F32, isOutput=False)
    binb = nc.declare_dram_parameter("binb", [128, 2], F32, isOutput=False)
    decb = nc.declare_dram_parameter("decb", [4, 1], F32, isOutput=False)
    out = nc.declare_dram_parameter("out", [1, n_rows], F32, isOutput=True)

    def mm(o, lhsT, rhs, start, stop):
        nc.tensor.matmul(o, lhsT, rhs, start=start, stop=stop)

    with tile.TileContext(nc, pool_alloc_mode="queue") as tc:
        with (
            tc.tile_pool(name="wpool", bufs=1) as wp,
            tc.tile_pool(name="io", bufs=4) as io,
            tc.tile_pool(name="acts", bufs=4) as ap,
            tc.tile_pool(name="psum", bufs=5, space="PSUM") as pp,
            tc.tile_pool(name="psumn", bufs=2, space="PSUM") as ppn,
            tc.tile_pool(name="psumd", bufs=1, space="PSUM") as ppd,
        ):
            encT_s = wp.tile([128, 256], BF16, name="encT_s", tag="encT_s")
            fobsT_s = wp.tile([128, 512], BF16, name="fobsT_s", tag="fobsT_s")
            wf1T_s = wp.tile([128, 1536], BF16, name="wf1T_s", tag="wf1T_s")
            whhT_s = wp.tile([128, 1536], BF16, name="whhT_s", tag="whhT_s")
            wihT_s = wp.tile([128, 1536], BF16, name="wihT_s", tag="wihT_s")
            decT_s = wp.tile([128, 2], BF16, name="decT_s", tag="decT_s")
            encb_s = wp.tile([128, 2], F32, name="encb_s", tag="encb_s")
            fobsb_s = wp.tile([128, 2], F32, name="fobsb_s", tag="fobsb_s")
            brz1_s = wp.tile([128, 4], F32, name="brz1_s", tag="brz1_s")
            bhn1_s = wp.tile([128, 2], F32, name="bhn1_s", tag="bhn1_s")
            brz2_s = wp.tile([128, 4], F32, name="brz2_s", tag="brz2_s")
            bhn2_s = wp.tile([128, 2], F32, name="bhn2_s", tag="bhn2_s")
            binb_s = wp.tile([128, 2], F32, name="binb_s", tag="binb_s")
            decb_s = wp.tile([4, 1], F32, name="decb_s", tag="decb_s")
            def dma_weights(pairs):
                for t, d in pairs:
                    nc.sync.dma_start(t[:], d.ap())

            xT_ap = xT.ap()
            out_ap = out.ap()

            st = {}

            def emitA1(t):
                # enc: e = relu(enc_W @ x + enc_b)  (relu+bias on Vector)
                r0 = t * R
                xt = io.tile([128, R], BF16, name="xt", tag="xt")
                nc.sync.dma_start(xt[:], xT_ap[:, r0 : r0 + R])
                pe = [pp.tile([128, 512], F32, name=f"pe{m}", tag="ps") for m in (0, 1)]
                for m in (0, 1):
                    mm(pe[m][:], encT_s[:, 128 * m : 128 * m + 128], xt[:], True, True)
                e = ap.tile([128, 1024], BF16, name="e", tag="e")
                # chunk 0 on Scalar (ACT relu+bias); chunk 1 alternates
                # Scalar/Vector per tile to fine-balance the two drains
                nc.scalar.activation(
                    e[:, S0], pe[0][:], AF.Relu, bias=encb_s[:, 0:1]
                )
                if t % 2 == 0:
                    nc.vector.tensor_scalar(
                        e[:, S1], pe[1][:], encb_s[:, 1:2], 0.0, OP.add, OP.max,
                    )
                else:
                    nc.scalar.activation(
                        e[:, S1], pe[1][:], AF.Relu, bias=encb_s[:, 1:2]
                    )
                st[t] = {"e": e}

            def emitA3a(t):
                # GRU1 gates from e via folded weights:
                #   prz1/pn1 = (W_hh @ fobs_W) @ e  [+ folded biases in ACT]
                e = st[t]["e"]
                prz = [pp.tile([128, 512], F32, name=f"prz{g}", tag="ps") for g in range(4)]
                for g in range(4):
                    for k in (0, 1):
                        mm(
                            prz[g][:],
                            wf1T_s[:, 768 * k + 128 * g : 768 * k + 128 * g + 128],
                            e[:, 512 * k : 512 * k + 512],
                            k == 0,
                            k == 1,
                        )
                pn = [ppn.tile([128, 512], F32, name=f"pn{m}", tag="psn") for m in (0, 1)]
                for m in (0, 1):
                    for k in (0, 1):
                        mm(
                            pn[m][:],
                            wf1T_s[:, 768 * k + 512 + 128 * m : 768 * k + 640 + 128 * m],
                            e[:, 512 * k : 512 * k + 512],
                            k == 0,
                            k == 1,
                        )
                rz1 = ap.tile([128, 2048], BF16, name="rz1", tag="rz1")
                for g in range(4):
                    nc.scalar.activation(
                        rz1[:, 512 * g : 512 * g + 512],
                        prz[g][:],
                        AF.Sigmoid,
                        bias=brz1_s[:, g : g + 1],
                    )
                # n1 = tanh(b_ih_n + r1 * (gh1_n + bhn1))
                tmp1 = ap.tile([128, 1024], BF16, name="tmp1", tag="tmp1")
                for m, sl in ((0, S0), (1, S1)):
                    nc.vector.scalar_tensor_tensor(
                        tmp1[:, sl], pn[m][:], bhn1_s[:, m : m + 1],
                        rz1[:, sl], OP.add, OP.mult,
                    )
                n1 = ap.tile([128, 1024], BF16, name="n1", tag="n1")
                for m, sl in ((0, S0), (1, S1)):
                    nc.scalar.activation(
                        n1[:, sl], tmp1[:, sl], AF.Tanh, bias=binb_s[:, m : m + 1]
                    )
                st[t]["rz1"] = rz1
                st[t]["n1"] = n1

            def emitA3b(t):
                # fobs (h0 stays in PSUM), then the GRU1 h-update + comm
                e = st[t].pop("e")
                rz1 = st[t].pop("rz1")
                n1 = st[t].pop("n1")
                ph = [pp.tile([128, 512], F32, name=f"ph{m}", tag="ps") for m in (0, 1)]
                for m in (0, 1):
                    for k in (0, 1):
                        mm(
                            ph[m][:],
                            fobsT_s[:, 256 * k + 128 * m : 256 * k + 128 * m + 128],
                            e[:, 512 * k : 512 * k + 512],
                            k == 0,
                            k == 1,
                        )
                # d1 = (h0 + fobs_b) - n1   (drains ph)
                d1 = ap.tile([128, 1024], BF16, name="d1", tag="d1")
                for m, sl in ((0, S0), (1, S1)):
                    nc.vector.scalar_tensor_tensor(
                        d1[:, sl], ph[m][:], fobsb_s[:, m : m + 1],
                        n1[:, sl], OP.add, OP.subtract,
                    )
                m1 = ap.tile([128, 1024], BF16, name="m1", tag="m1")
                nc.gpsimd.tensor_mul(m1[:], rz1[:, 1024:2048], d1[:])
                h1 = ap.tile([128, 1024], BF16, name="h1", tag="h1", bufs=6)
                nc.vector.tensor_add(h1[:], n1[:], m1[:])
                # comm: cp = (sum_group h1) - h1  (1/A folded into W_ih)
                S = ap.tile([128, 32], F32, name="S", tag="S")
                nc.vector.tensor_reduce(
                    S[:],
                    h1[:].rearrange("p (s a) -> p s a", a=32),
                    mybir.AxisListType.X,
                    OP.add,
                )
                cp = ap.tile([128, 1024], BF16, name="cp", tag="cp", bufs=6)
                Sb = S[:].unsqueeze(-1).broadcast_to([128, 32, 32])
                nc.gpsimd.tensor_tensor(
                    cp[:].rearrange("p (s a) -> p s a", a=32),
                    Sb,
                    h1[:].rearrange("p (s a) -> p s a", a=32),
                    OP.subtract,
                )
                st[t]["h1"] = h1
                st[t]["cp"] = cp

            def emitBrz(t):
                # GRU2 r,z gates: gi + gh accumulated in one PSUM group
                h1, cp = st[t]["h1"], st[t]["cp"]
                prz2 = [pp.tile([128, 512], F32, name=f"prz2{g}", tag="ps") for g in range(4)]
                for g in range(4):
                    w0 = 128 * g
                    mm(prz2[g][:], wihT_s[:, w0 : w0 + 128], cp[:, S0], True, False)
                    mm(prz2[g][:], wihT_s[:, 768 + w0 : 768 + w0 + 128], cp[:, S1], False, False)
                    mm(prz2[g][:], whhT_s[:, w0 : w0 + 128], h1[:, S0], False, False)
                    mm(prz2[g][:], whhT_s[:, 768 + w0 : 768 + w0 + 128], h1[:, S1], False, True)
                rz2 = ap.tile([128, 2048], BF16, name="rz2", tag="rz2")
                for g in range(4):
                    nc.scalar.activation(
                        rz2[:, 512 * g : 512 * g + 512],
                        prz2[g][:],
                        AF.Sigmoid,
                        bias=brz2_s[:, g : g + 1],
                    )
                st[t]["rz2"] = rz2

            def emitBn(t):
                # GRU2 n gate + h2
                h1, cp, rz2 = st[t]["h1"], st[t]["cp"], st[t]["rz2"]
                phn = [pp.tile([128, 512], F32, name=f"phn{m}", tag="ps") for m in (0, 1)]
                pin = [pp.tile([128, 512], F32, name=f"pin{m}", tag="ps") for m in (0, 1)]
                for m in (0, 1):
                    for k in (0, 1):
                        mm(
                            phn[m][:],
                            whhT_s[:, 768 * k + 512 + 128 * m : 768 * k + 640 + 128 * m],
                            h1[:, 512 * k : 512 * k + 512],
                            k == 0,
                            k == 1,
                        )
                for m in (0, 1):
                    for k in (0, 1):
                        mm(
                            pin[m][:],
                            wihT_s[:, 768 * k + 512 + 128 * m : 768 * k + 640 + 128 * m],
                            cp[:, 512 * k : 512 * k + 512],
                            k == 0,
                            k == 1,
                        )
                # n2 = tanh(b_ih_n + i_n + r2 * (gh_n + b_hh_n))
                tmp2 = ap.tile([128, 1024], BF16, name="tmp2", tag="tmp2")
                for m, sl in ((0, S0), (1, S1)):
                    nc.vector.scalar_tensor_tensor(
                        tmp2[:, sl], phn[m][:], bhn2_s[:, m : m + 1],
                        rz2[:, sl], OP.add, OP.mult,
                    )
                s2 = ap.tile([128, 1024], BF16, name="s2", tag="s2")
                for m, sl in ((0, S0), (1, S1)):
                    nc.vector.tensor_add(s2[:, sl], tmp2[:, sl], pin[m][:])
                n2 = ap.tile([128, 1024], BF16, name="n2", tag="n2")
                for m, sl in ((0, S0), (1, S1)):
                    nc.scalar.activation(
                        n2[:, sl], s2[:, sl], AF.Tanh, bias=binb_s[:, m : m + 1]
                    )
                # h2 = n2 + z2*(h1 - n2); the final tile's chain runs with no
                # other tiles overlapping, so keep it on the faster engine
                eng = nc.vector if t == nt - 1 else nc.gpsimd
                d2 = ap.tile([128, 1024], BF16, name="d2", tag="d2")
                nc.vector.tensor_sub(d2[:], h1[:], n2[:])
                m2 = ap.tile([128, 1024], BF16, name="m2", tag="m2")
                eng.tensor_mul(m2[:], rz2[:, 1024:2048], d2[:])
                h2 = ap.tile([128, 1024], BF16, name="h2", tag="h2", bufs=6)
                eng.tensor_add(h2[:], n2[:], m2[:])
                st[t]["h2"] = h2

            def emitC(t):
                h2 = st.pop(t)["h2"]
                r0 = t * R
                pd = ppd.tile([1, 512], F32, name="pd", tag="psd")
                mm(pd[:], decT_s[:, 0:1], h2[:, S0], True, False)
                mm(pd[:], decT_s[:, 1:2], h2[:, S1], False, True)
                ot = io.tile([1, 512], F32, name="ot", tag="ot")
                nc.scalar.activation(
                    ot[:], pd[:], AF.Identity, bias=decb_s[0:1, 0:1]
                )
                nc.sync.dma_start(out_ap[0:1, r0 : r0 + R], ot[:])

            def emitA(t):
                emitA1(t)
                emitA3a(t)
                emitA3b(t)

            LA = 3  # pipeline lookahead (tiles)
            assert nt % 4 == 0
            # weight DMAs in dependency order, interleaved with the prologue,
            # so the first matmuls aren't queued behind all 14 transfers
            dma_weights([(encT_s, encT), (encb_s, encb)])
            emitA1(0)
            dma_weights([
                (wf1T_s, wf1T), (brz1_s, brz1), (bhn1_s, bhn1), (binb_s, binb),
            ])
            emitA3a(0)
            dma_weights([(fobsT_s, fobsT), (fobsb_s, fobsb)])
            emitA3b(0)
            dma_weights([
                (whhT_s, whhT), (wihT_s, wihT), (brz2_s, brz2),
                (bhn2_s, bhn2), (decT_s, decT), (decb_s, decb),
            ])
            for i in range(1, min(LA + 1, nt)):
                emitA(i)
            emitBrz(0)
            emitBn(0)
            for t in range(1, nt):
                # B(t) fully before A(t+LA): keeps tile t's tail ops ahead of
                # tile t+LA's producers in each engine FIFO
                emitBrz(t)
                emitBn(t)
                if t + LA < nt:
                    emitA(t + LA)
                if t >= 1:
                    emitC(t - 1)
            emitC(nt - 1)

    nc.compile()
    return nc


def prep_shared(enc_W, enc_b, fobs_W, fobs_b, W_ih, b_ih, W_hh, b_hh, dec_W, dec_b):
    f = np.float32
    enc_W = np.asarray(enc_W, f)
    enc_b = np.asarray(enc_b, f)
    fobs_W = np.asarray(fobs_W, f)
    fobs_b = np.asarray(fobs_b, f)
    W_ih = np.asarray(W_ih, f)
    b_ih = np.asarray(b_ih, f)
    W_hh = np.asarray(W_hh, f)
    b_hh = np.asarray(b_hh, f)
    whhT = W_hh.T                                # [256, 768]
    wihT = (W_ih / A).T                          # [256, 768], 1/A folded in
    wf1 = (W_hh @ fobs_W).T                      # [256, 768] folded GRU1 gates
    gh1b = W_hh @ fobs_b + b_hh                  # [768] folded GRU1 gate bias
    bsum = b_ih + b_hh
    bf = NP_BF16

    def two(x):   # [256, C] -> [128, 2C] contraction-chunked layout
        return np.ascontiguousarray(
            np.concatenate([x[0:128], x[128:256]], axis=1)
        ).astype(bf)

    return {
        "encT": np.ascontiguousarray(enc_W.T).astype(bf),        # [128,256]
        "fobsT": two(fobs_W.T),                                  # [128,512]
        "wf1T": two(wf1),                                        # [128,1536]
        "whhT": two(whhT),                                       # [128,1536]
        "wihT": two(wihT),                                       # [128,1536]
        "decT": two(dec_W.T),                                    # [128,2]
        "encb": np.ascontiguousarray(enc_b.reshape(2, 128).T.astype(f)),
        "fobsb": np.ascontiguousarray(fobs_b.reshape(2, 128).T.astype(f)),
        "brz1": np.ascontiguousarray(
            (b_ih[0:512] + gh1b[0:512]).reshape(4, 128).T.astype(f)
        ),
        "bhn1": np.ascontiguousarray(gh1b[512:768].reshape(2, 128).T.astype(f)),
        "brz2": np.ascontiguousarray(bsum[0:512].reshape(4, 128).T.astype(f)),
        "bhn2": np.ascontiguousarray(b_hh[512:768].reshape(2, 128).T.astype(f)),
        "binb": np.ascontiguousarray(b_ih[512:768].reshape(2, 128).T.astype(f)),
        "decb": np.full((4, 1), np.asarray(dec_b, f).reshape(()), dtype=f),
    }


_NC_CACHE = {}


def _get_nc(n_rows):
    if n_rows not in _NC_CACHE:
        _NC_CACHE[n_rows] = build_nc(n_rows)
    return _NC_CACHE[n_rows]


def run(inputs, trace=False):
    """Shard, run on 8 cores, gather. Returns (out [B,A,1] f32, results)."""
    obs = np.asarray(inputs["obs"], dtype=np.float32)
    shared = prep_shared(
        np.asarray(inputs["enc_W"]), np.asarray(inputs["enc_b"]),
        np.asarray(inputs["fobs_W"]), np.asarray(inputs["fobs_b"]),
        np.asarray(inputs["W_ih"]), np.asarray(inputs["b_ih"]),
        np.asarray(inputs["W_hh"]), np.asarray(inputs["b_hh"]),
        np.asarray(inputs["dec_W"]), np.asarray(inputs["dec_b"]),
    )
    in_maps = []
    for c in range(NCORES):
        xT = np.ascontiguousarray(
            obs[c * B_LOC : (c + 1) * B_LOC].reshape(N_LOC, D).T
        ).astype(NP_BF16)
        in_maps.append({"xT": xT, **shared})

    nc = _get_nc(N_LOC)
    res = run_bass_kernel_spmd(nc, in_maps, core_ids=list(range(NCORES)), trace=trace)
    outs = [res.results[c]["out"].reshape(N_LOC) for c in range(NCORES)]
    full = np.concatenate(outs).reshape(B, A, 1).astype(np.float32)
    return full, res


def kernel(**inputs):
    out, _ = run(inputs, trace=False)
    return out


# revision 24
# speedup vs baseline: 1.0987x; 1.0987x over previous
"""CommNet critic forward kernel for 8 trn2 NeuronCores.

Sharding: pure data parallel over the batch dim (B=2048 -> 256 per core).
Weights (<1MB) replicated. The agent-mean communication is within each
sample's 32-agent group, which never crosses a core boundary, so there are
no collectives.

v2 vs v1: the engine loads are rebalanced around the measured busy times
(PE ~150us, Scalar ACT 194us, Vector 168us, GpSimd 114us on a 240us span):
  * fobs is folded into GRU1 host-side (Wfold1 = W_hh @ fobs_W), so the
    GRU1 gate matmuls read the encoder output e directly and h0 never
    needs a dedicated PSUM drain: its bias-add merges into the d1 STT.
  * Scalar keeps only the transcendentals (sigmoid/tanh) plus the tiny
    dec drain; relu moves to Vector (tensor_scalar add+max), and the
    GRU h-updates are split Vector/GpSimd so no engine exceeds ~130us.
  * PE stream (44 matmuls/tile, all N=512 bf16) is the binding floor;
    the emit order interleaves A(t+2) work into B(t) so it never stalls.
"""

import sys

sys.path.insert(0, "/opt/trn_rl_repo")

import ml_dtypes
import numpy as np

import concourse.bacc as bacc
import concourse.mybir as mybir
import concourse.tile as tile
from concourse.bass_utils import run_bass_kernel_spmd

B, A, D, H = 2048, 32, 128, 256
NCORES = 8
B_LOC = B // NCORES          # 256 samples per core
N_LOC = B_LOC * A            # 8192 rows per core
R = 512                      # rows per tile (one PSUM bank of fp32)

F32 = mybir.dt.float32
BF16 = mybir.dt.bfloat16
NP_BF16 = ml_dtypes.bfloat16

AF = mybir.ActivationFunctionType
OP = mybir.AluOpType

S0 = slice(0, 512)
S1 = slice(512, 1024)


def build_nc(n_rows=N_LOC):
    assert n_rows % R == 0
    nt = n_rows // R
    nc = bacc.Bacc("TRN2", target_bir_lowering=False, debug=False)

    xT = nc.declare_dram_parameter("xT", [D, n_rows], BF16, isOutput=False)
    encT = nc.declare_dram_parameter("encT", [128, 256], BF16, isOutput=False)
    fobsT = nc.declare_dram_parameter("fobsT", [128, 512], BF16, isOutput=False)
    wf1T = nc.declare_dram_parameter("wf1T", [128, 1536], BF16, isOutput=False)
    whhT = nc.declare_dram_parameter("whhT", [128, 1536], BF16, isOutput=False)
    wihT = nc.declare_dram_parameter("wihT", [128, 1536], BF16, isOutput=False)
    decT = nc.declare_dram_parameter("decT", [128, 2], BF16, isOutput=False)
    encb = nc.declare_dram_parameter("encb", [128, 2], F32, isOutput=False)
    fobsb = nc.declare_dram_parameter("fobsb", [128, 2], F32, isOutput=False)
    brz1 = nc.declare_dram_parameter("brz1", [128, 4], F32, isOutput=False)
    bhn1 = nc.declare_dram_parameter("bhn1", [128, 2], F32, isOutput=False)
    brz2 = nc.declare_dram_parameter("brz2", [128, 4], F32, isOutput=False)
    bhn2 = nc.declare_dram_parameter("bhn2", [128, 2], F32, isOutput=False)
    binb = nc.declare_dram_parameter("binb", [128, 2], F32, isOutput=False)
    decb = nc.declare_dram_parameter("decb", [4, 1], F32, isOutput=False)
    out = nc.declare_dram_parameter("out", [1, n_rows], F32, isOutput=True)

    def mm(o, lhsT, rhs, start, stop):
        nc.tensor.matmul(o, lhsT, rhs, start=start, stop=stop)

    with tile.TileContext(nc, pool_alloc_mode="queue") as tc:
        with (
            tc.tile_pool(name="wpool", bufs=1) as wp,
            tc.tile_pool(name="io", bufs=4) as io,
            tc.tile_pool(name="acts", bufs=4) as ap,
            tc.tile_pool(name="psum", bufs=5, space="PSUM") as pp,
            tc.tile_pool(name="psumn", bufs=2, space="PSUM") as ppn,
            tc.tile_pool(name="psumd", bufs=1, space="PSUM") as ppd,
        ):
            encT_s = wp.tile([128, 256], BF16, name="encT_s", tag="encT_s")
            fobsT_s = wp.tile([128, 512], BF16, name="fobsT_s", tag="fobsT_s")
            wf1T_s = wp.tile([128, 1536], BF16, name="wf1T_s", tag="wf1T_s")
            whhT_s = wp.tile([128, 1536], BF16, name="whhT_s", tag="whhT_s")
            wihT_s = wp.tile([128, 1536], BF16, name="wihT_s", tag="wihT_s")
            decT_s = wp.tile([128, 2], BF16, name="decT_s", tag="decT_s")
            encb_s = wp.tile([128, 2], F32, name="encb_s", tag="encb_s")
            fobsb_s = wp.tile([128, 2], F32, name="fobsb_s", tag="fobsb_s")
            brz1_s = wp.tile([128, 4], F32, name="brz1_s", tag="brz1_s")
            bhn1_s = wp.tile([128, 2], F32, name="bhn1_s", tag="bhn1_s")
            brz2_s = wp.tile([128, 4], F32, name="brz2_s", tag="brz2_s")
            bhn2_s = wp.tile([128, 2], F32, name="bhn2_s", tag="bhn2_s")
            binb_s = wp.tile([128, 2], F32, name="binb_s", tag="binb_s")
            decb_s = wp.tile([4, 1], F32, name="decb_s", tag="decb_s")
            def dma_weights(pairs):
                for t, d in pairs:
                    nc.sync.dma_start(t[:], d.ap())

            xT_ap = xT.ap()
            out_ap = out.ap()

            st = {}

            def emitA1(t):
                # enc: e = relu(enc_W @ x + enc_b)  (relu+bias on Vector)
                r0 = t * R
                xt = io.tile([128, R], BF16, name="xt", tag="xt")
                nc.sync.dma_start(xt[:], xT_ap[:, r0 : r0 + R])
                pe = [pp.tile([128, 512], F32, name=f"pe{m}", tag="ps") for m in (0, 1)]
                for m in (0, 1):
                    mm(pe[m][:], encT_s[:, 128 * m : 128 * m + 128], xt[:], True, True)
                e = ap.tile([128, 1024], BF16, name="e", tag="e")
                # chunk 0 on Scalar (ACT relu+bias), chunk 1 on Vector: splits
                # the drain across the two PSUM-capable engines
                nc.scalar.activation(
                    e[:, S0], pe[0][:], AF.Relu, bias=encb_s[:, 0:1]
                )
                nc.vector.tensor_scalar(
                    e[:, S1], pe[1][:], encb_s[:, 1:2], 0.0, OP.add, OP.max,
                )
                st[t] = {"e": e}

            def emitA3a(t):
                # GRU1 gates from e via folded weights:
                #   prz1/pn1 = (W_hh @ fobs_W) @ e  [+ folded biases in ACT]
                e = st[t]["e"]
                prz = [pp.tile([128, 512], F32, name=f"prz{g}", tag="ps") for g in range(4)]
                for g in range(4):
                    for k in (0, 1):
                        mm(
                            prz[g][:],
                            wf1T_s[:, 768 * k + 128 * g : 768 * k + 128 * g + 128],
                            e[:, 512 * k : 512 * k + 512],
                            k == 0,
                            k == 1,
                        )
                pn = [ppn.tile([128, 512], F32, name=f"pn{m}", tag="psn") for m in (0, 1)]
                for m in (0, 1):
                    for k in (0, 1):
                        mm(
                            pn[m][:],
                            wf1T_s[:, 768 * k + 512 + 128 * m : 768 * k + 640 + 128 * m],
                            e[:, 512 * k : 512 * k + 512],
                            k == 0,
                            k == 1,
                        )
                rz1 = ap.tile([128, 2048], BF16, name="rz1", tag="rz1")
                for g in range(4):
                    nc.scalar.activation(
                        rz1[:, 512 * g : 512 * g + 512],
                        prz[g][:],
                        AF.Sigmoid,
                        bias=brz1_s[:, g : g + 1],
                    )
                # n1 = tanh(b_ih_n + r1 * (gh1_n + bhn1))
                tmp1 = ap.tile([128, 1024], BF16, name="tmp1", tag="tmp1")
                for m, sl in ((0, S0), (1, S1)):
                    nc.vector.scalar_tensor_tensor(
                        tmp1[:, sl], pn[m][:], bhn1_s[:, m : m + 1],
                        rz1[:, sl], OP.add, OP.mult,
                    )
                n1 = ap.tile([128, 1024], BF16, name="n1", tag="n1")
                for m, sl in ((0, S0), (1, S1)):
                    nc.scalar.activation(
                        n1[:, sl], tmp1[:, sl], AF.Tanh, bias=binb_s[:, m : m + 1]
                    )
                st[t]["rz1"] = rz1
                st[t]["n1"] = n1

            def emitA3b(t):
                # fobs (h0 stays in PSUM), then the GRU1 h-update + comm
                e = st[t].pop("e")
                rz1 = st[t].pop("rz1")
                n1 = st[t].pop("n1")
                ph = [pp.tile([128, 512], F32, name=f"ph{m}", tag="ps") for m in (0, 1)]
                for m in (0, 1):
                    for k in (0, 1):
                        mm(
                            ph[m][:],
                            fobsT_s[:, 256 * k + 128 * m : 256 * k + 128 * m + 128],
                            e[:, 512 * k : 512 * k + 512],
                            k == 0,
                            k == 1,
                        )
                # d1 = (h0 + fobs_b) - n1   (drains ph)
                d1 = ap.tile([128, 1024], BF16, name="d1", tag="d1")
                for m, sl in ((0, S0), (1, S1)):
                    nc.vector.scalar_tensor_tensor(
                        d1[:, sl], ph[m][:], fobsb_s[:, m : m + 1],
                        n1[:, sl], OP.add, OP.subtract,
                    )
                m1 = ap.tile([128, 1024], BF16, name="m1", tag="m1")
                nc.vector.tensor_mul(m1[:], rz1[:, 1024:2048], d1[:])
                h1 = ap.tile([128, 1024], BF16, name="h1", tag="h1", bufs=6)
                nc.vector.tensor_add(h1[:], n1[:], m1[:])
                # comm: cp = (sum_group h1) - h1  (1/A folded into W_ih)
                S = ap.tile([128, 32], F32, name="S", tag="S")
                nc.vector.tensor_reduce(
                    S[:],
                    h1[:].rearrange("p (s a) -> p s a", a=32),
                    mybir.AxisListType.X,
                    OP.add,
                )
                cp = ap.tile([128, 1024], BF16, name="cp", tag="cp", bufs=6)
                Sb = S[:].unsqueeze(-1).broadcast_to([128, 32, 32])
                nc.gpsimd.tensor_tensor(
                    cp[:].rearrange("p (s a) -> p s a", a=32),
                    Sb,
                    h1[:].rearrange("p (s a) -> p s a", a=32),
                    OP.subtract,
                )
                st[t]["h1"] = h1
                st[t]["cp"] = cp

            def emitBrz(t):
                # GRU2 r,z gates: gi + gh accumulated in one PSUM group
                h1, cp = st[t]["h1"], st[t]["cp"]
                prz2 = [pp.tile([128, 512], F32, name=f"prz2{g}", tag="ps") for g in range(4)]
                for g in range(4):
                    w0 = 128 * g
                    mm(prz2[g][:], wihT_s[:, w0 : w0 + 128], cp[:, S0], True, False)
                    mm(prz2[g][:], wihT_s[:, 768 + w0 : 768 + w0 + 128], cp[:, S1], False, False)
                    mm(prz2[g][:], whhT_s[:, w0 : w0 + 128], h1[:, S0], False, False)
                    mm(prz2[g][:], whhT_s[:, 768 + w0 : 768 + w0 + 128], h1[:, S1], False, True)
                rz2 = ap.tile([128, 2048], BF16, name="rz2", tag="rz2")
                for g in range(4):
                    nc.scalar.activation(
                        rz2[:, 512 * g : 512 * g + 512],
                        prz2[g][:],
                        AF.Sigmoid,
                        bias=brz2_s[:, g : g + 1],
                    )
                st[t]["rz2"] = rz2

            def emitBn(t):
                # GRU2 n gate + h2
                h1, cp, rz2 = st[t]["h1"], st[t]["cp"], st[t]["rz2"]
                phn = [pp.tile([128, 512], F32, name=f"phn{m}", tag="ps") for m in (0, 1)]
                pin = [pp.tile([128, 512], F32, name=f"pin{m}", tag="ps") for m in (0, 1)]
                for m in (0, 1):
                    for k in (0, 1):
                        mm(
                            phn[m][:],
                            whhT_s[:, 768 * k + 512 + 128 * m : 768 * k + 640 + 128 * m],
                            h1[:, 512 * k : 512 * k + 512],
                            k == 0,
                            k == 1,
                        )
                for m in (0, 1):
                    for k in (0, 1):
                        mm(
                            pin[m][:],
                            wihT_s[:, 768 * k + 512 + 128 * m : 768 * k + 640 + 128 * m],
                            cp[:, 512 * k : 512 * k + 512],
                            k == 0,
                            k == 1,
                        )
                # n2 = tanh(b_ih_n + i_n + r2 * (gh_n + b_hh_n))
                tmp2 = ap.tile([128, 1024], BF16, name="tmp2", tag="tmp2")
                for m, sl in ((0, S0), (1, S1)):
                    nc.vector.scalar_tensor_tensor(
                        tmp2[:, sl], phn[m][:], bhn2_s[:, m : m + 1],
                        rz2[:, sl], OP.add, OP.mult,
                    )
                s2 = ap.tile([128, 1024], BF16, name="s2", tag="s2")
                for m, sl in ((0, S0), (1, S1)):
                    nc.vector.tensor_add(s2[:, sl], tmp2[:, sl], pin[m][:])
                n2 = ap.tile([128, 1024], BF16, name="n2", tag="n2")
                for m, sl in ((0, S0), (1, S1)):
                    nc.scalar.activation(
                        n2[:, sl], s2[:, sl], AF.Tanh, bias=binb_s[:, m : m + 1]
                    )
                # h2 = n2 + z2*(h1 - n2); the final tile's chain runs with no
                # other tiles overlapping, so keep it on the faster engine
                eng = nc.vector if t == nt - 1 else nc.gpsimd
                d2 = ap.tile([128, 1024], BF16, name="d2", tag="d2")
                nc.vector.tensor_sub(d2[:], h1[:], n2[:])
                m2 = ap.tile([128, 1024], BF16, name="m2", tag="m2")
                eng.tensor_mul(m2[:], rz2[:, 1024:2048], d2[:])
                h2 = ap.tile([128, 1024], BF16, name="h2", tag="h2", bufs=6)
                eng.tensor_add(h2[:], n2[:], m2[:])
                st[t]["h2"] = h2

            def emitC(t):
                h2 = st.pop(t)["h2"]
                r0 = t * R
                pd = ppd.tile([1, 512], F32, name="pd", tag="psd")
                mm(pd[:], decT_s[:, 0:1], h2[:, S0], True, False)
                mm(pd[:], decT_s[:, 1:2], h2[:, S1], False, True)
                ot = io.tile([1, 512], F32, name="ot", tag="ot")
                nc.scalar.activation(
                    ot[:], pd[:], AF.Identity, bias=decb_s[0:1, 0:1]
                )
                nc.sync.dma_start(out_ap[0:1, r0 : r0 + R], ot[:])

            def emitA(t):
                emitA1(t)
                emitA3a(t)
                emitA3b(t)

            LA = 3  # pipeline lookahead (tiles)
            assert nt % 4 == 0
            # weight DMAs in dependency order, interleaved with the prologue,
            # so the first matmuls aren't queued behind all 14 transfers
            dma_weights([(encT_s, encT), (encb_s, encb)])
            emitA1(0)
            dma_weights([
                (wf1T_s, wf1T), (brz1_s, brz1), (bhn1_s, bhn1), (binb_s, binb),
            ])
            emitA3a(0)
            dma_weights([(fobsT_s, fobsT), (fobsb_s, fobsb)])
            emitA3b(0)
            dma_weights([
                (whhT_s, whhT), (wihT_s, wihT), (brz2_s, brz2),
                (bhn2_s, bhn2), (decT_s, decT), (decb_s, decb),
            ])
            for i in range(1, min(LA + 1, nt)):
                emitA(i)
            emitBrz(0)
            emitBn(0)
            for t in range(1, nt):
                # B(t) fully before A(t+LA): keeps tile t's tail ops ahead of
                # tile t+LA's producers in each engine FIFO
                emitBrz(t)
                emitBn(t)
                if t + LA < nt:
                    emitA(t + LA)
                if t >= 1:
                    emitC(t - 1)
            emitC(nt - 1)

    nc.compile()
    return nc


def prep_shared(enc_W, enc_b, fobs_W, fobs_b, W_ih, b_ih, W_hh, b_hh, dec_W, dec_b):
    f = np.float32
    enc_W = np.asarray(enc_W, f)
    enc_b = np.asarray(enc_b, f)
    fobs_W = np.asarray(fobs_W, f)
    fobs_b = np.asarray(fobs_b, f)
    W_ih = np.asarray(W_ih, f)
    b_ih = np.asarray(b_ih, f)
    W_hh = np.asarray(W_hh, f)
    b_hh = np.asarray(b_hh, f)
    whhT = W_hh.T                                # [256, 768]
    wihT = (W_ih / A).T                          # [256, 768], 1/A folded in
    wf1 = (W_hh @ fobs_W).T                      # [256, 768] folded GRU1 gates
    gh1b = W_hh @ fobs_b + b_hh                  # [768] folded GRU1 gate bias
    bsum = b_ih + b_hh
    bf = NP_BF16

    def two(x):   # [256, C] -> [128, 2C] contraction-chunked layout
        return np.ascontiguousarray(
            np.concatenate([x[0:128], x[128:256]], axis=1)
        ).astype(bf)

    return {
        "encT": np.ascontiguousarray(enc_W.T).astype(bf),        # [128,256]
        "fobsT": two(fobs_W.T),                                  # [128,512]
        "wf1T": two(wf1),                                        # [128,1536]
        "whhT": two(whhT),                                       # [128,1536]
        "wihT": two(wihT),                                       # [128,1536]
        "decT": two(dec_W.T),                                    # [128,2]
        "encb": np.ascontiguousarray(enc_b.reshape(2, 128).T.astype(f)),
        "fobsb": np.ascontiguousarray(fobs_b.reshape(2, 128).T.astype(f)),
        "brz1": np.ascontiguousarray(
            (b_ih[0:512] + gh1b[0:512]).reshape(4, 128).T.astype(f)
        ),
        "bhn1": np.ascontiguousarray(gh1b[512:768].reshape(2, 128).T.astype(f)),
        "brz2": np.ascontiguousarray(bsum[0:512].reshape(4, 128).T.astype(f)),
        "bhn2": np.ascontiguousarray(b_hh[512:768].reshape(2, 128).T.astype(f)),
        "binb": np.ascontiguousarray(b_ih[512:768].reshape(2, 128).T.astype(f)),
        "decb": np.full((4, 1), np.asarray(dec_b, f).reshape(()), dtype=f),
    }


_NC_CACHE = {}


def _get_nc(n_rows):
    if n_rows not in _NC_CACHE:
        _NC_CACHE[n_rows] = build_nc(n_rows)
    return _NC_CACHE[n_rows]


def run(inputs, trace=False):
    """Shard, run on 8 cores, gather. Returns (out [B,A,1] f32, results)."""
    obs = np.asarray(inputs["obs"], dtype=np.float32)
    shared = prep_shared(
        np.asarray(inputs["enc_W"]), np.asarray(inputs["enc_b"]),
        np.asarray(inputs["fobs_W"]), np.asarray(inputs["fobs_b"]),
        np.asarray(inputs["W_ih"]), np.asarray(inputs["b_ih"]),
        np.asarray(inputs["W_hh"]), np.asarray(inputs["b_hh"]),
        np.asarray(inputs["dec_W"]), np.asarray(inputs["dec_b"]),
    )
    in_maps = []
    for c in range(NCORES):
        xT = np.ascontiguousarray(
            obs[c * B_LOC : (c + 1) * B_LOC].reshape(N_LOC, D).T
        ).astype(NP_BF16)
        in_maps.append({"xT": xT, **shared})

    nc = _get_nc(N_LOC)
    res = run_bass_kernel_spmd(nc, in_maps, core_ids=list(range(NCORES)), trace=trace)
    outs = [res.results[c]["out"].reshape(N_LOC) for c in range(NCORES)]
    full = np.concatenate(outs).reshape(B, A, 1).astype(np.float32)
    return full, res


def kernel(**inputs):
    out, _ = run(inputs, trace=False)
    return out
